# revision 8
# baseline (speedup 1.0000x reference)
"""EdgeConv (PyG, aggr='max') Trainium2 kernel, 8-core SPMD.

Math: out_i = max_{e: dst(e)=i} relu(x_i @ W1.T + (x_src(e) - x_i) @ W2.T + b)
with W = [W1 | W2].  Rewriting:
    msg_e = relu(A_i + g_src(e)),  A = x @ (W1-W2).T + b,  g = x @ W2.T
Since A_i is constant within segment i and relu is monotone:
    out_i = relu(A_i + max_e g_src(e))
The reference's dst is repeat(arange(N), DEG) (fixed-degree graph), so
segments are 16 consecutive edges; segment-max becomes a grouped reduce.

Two SPMD launches on 8 cores, everything in bf16 (abs tolerance comfortably
allows it):
  L1 (node-parallel): per-core 6250-node shard computes g = x @ W2.T via one
     128-node-tile matmul per tile (x arrives host-transposed [65, 6400] with
     a trailing ones row so biases can be folded as an extra contraction row)
     and writes the bf16 g table in a t-major row permutation (HBM row
     r = 50*(l%128) + l//128) so every DMA runs full-size descriptors.
  L2 (edge-parallel): per-core 100352-edge shard. A = x @ (W1-W2).T + b is
     computed in-launch on the otherwise-idle PE. Chunks of node-tiles
     bulk-gather 256B bf16 row-PAIRS [g_{2r} | g_{2r+1}] by (row>>1) with
     dma_gather (int16 pair ids), parity-select via copy + copy_predicated,
     then a contiguous-halves max tree (exact in bf16), add A, relu, and a
     t-major bf16 output write the host unpermutes/upcasts. The gather DMA
     is the roofline (~1.42 ns/edge); 25 uniform 4096-edge chunks keep the
     hardware happy (varying num_idxs across dma_gathers wedges the device)
     while keeping the final select+reduce drain to ~5 us.
"""

import numpy as np

N_NODES = 50000
DEG = 16
C = 64
N_CORES = 8
NSH = N_NODES // N_CORES  # 6250 nodes per core
P = 128
NSH_PAD = 6400  # 50 * 128
NT = NSH_PAD // P  # 50 node tiles of 128
TCH = 2  # node-tiles per gather chunk (uniform: varying num_idxs wedges the hw)
NCHUNKS = NT // TCH  # 25
CHUNK_T = [TCH] * NCHUNKS
NPAIR = N_CORES * NSH_PAD // 2  # 25600 pair rows in the gather table
MMB = 5  # node-tiles per PSUM batch (A matmuls, launch 2)
DMB = 25  # node-tiles per PSUM batch (g matmuls, launch 1)
IDX_W = NSH_PAD * DEG // 16  # 6400 int16 per partition, all chunks

_cache = {}


def _build_dense():
    import concourse.bacc as bacc
    import concourse.mybir as mybir
    from concourse.tile import TileContext

    nc = bacc.Bacc("TRN2", target_bir_lowering=False, debug=False)
    bf16 = mybir.dt.bfloat16
    f32 = mybir.dt.float32
    xsT = nc.dram_tensor("xsT", [C + 1, NSH_PAD], bf16, kind="ExternalInput")
    v2 = nc.dram_tensor("v2", [C + 1, C], bf16, kind="ExternalInput")
    gout = nc.dram_tensor("gout", [NSH_PAD, C], bf16, kind="ExternalOutput")

    with TileContext(nc) as tc:
        with (
            tc.tile_pool(name="const", bufs=1) as cpool,
            tc.tile_pool(name="sbuf", bufs=2) as pool,
            tc.tile_pool(name="psum", bufs=2, space="PSUM") as psum,
        ):
            xsT_sb = cpool.tile([C + 1, NSH_PAD], bf16)
            half = DMB * P
            nc.sync.dma_start(out=xsT_sb[:, 0:half], in_=xsT[:, 0:half])
            nc.sync.dma_start(out=xsT_sb[:, half:], in_=xsT[:, half:])
            v2_sb = cpool.tile([C + 1, C], bf16)
            nc.sync.dma_start(out=v2_sb[:], in_=v2[:])
            for b in range(NT // DMB):
                ps = psum.tile([P, DMB, C], f32, tag="h")
                for t in range(DMB):
                    T = b * DMB + t
                    nc.tensor.matmul(
                        out=ps[:, t, :],
                        lhsT=xsT_sb[:, T * P : (T + 1) * P],
                        rhs=v2_sb[:],
                        start=True,
                        stop=True,
                    )
                g_sb = pool.tile([P, DMB, C], bf16, tag="g")
                nc.scalar.copy(out=g_sb[:], in_=ps[:])
                nc.sync.dma_start(
                    out=gout[:, :].rearrange("(p T) c -> p T c", p=P)[
                        :, b * DMB : (b + 1) * DMB, :
                    ],
                    in_=g_sb[:],
                )
    nc.compile()
    return nc


def _build_gather():
    import concourse.bacc as bacc
    import concourse.mybir as mybir
    from concourse.tile import TileContext

    nc = bacc.Bacc("TRN2", target_bir_lowering=False, debug=False)
    bf16 = mybir.dt.bfloat16
    f32 = mybir.dt.float32
    i16 = mybir.dt.int16
    u8 = mybir.dt.uint8
    T0 = TCH
    gpair = nc.dram_tensor("gpair", [NPAIR, 2 * C], bf16, kind="ExternalInput")
    idxs = nc.dram_tensor("idxs", [P, IDX_W], i16, kind="ExternalInput")
    msk = nc.dram_tensor("msk", [P, NT * DEG], u8, kind="ExternalInput")
    xsT = nc.dram_tensor("xsT", [C + 1, NSH_PAD], bf16, kind="ExternalInput")
    v1 = nc.dram_tensor("v1", [C + 1, C], bf16, kind="ExternalInput")
    osh = nc.dram_tensor("osh", [NSH_PAD, C], bf16, kind="ExternalOutput")

    with TileContext(nc) as tc:
        with (
            tc.tile_pool(name="const", bufs=1) as cpool,
            tc.tile_pool(name="sbuf", bufs=2) as pool,
            tc.tile_pool(name="gat", bufs=3) as gpool,
            tc.tile_pool(name="psum", bufs=2, space="PSUM") as psum,
        ):
            # chunk 0's wrapped indices first so the first gather's
            # descriptor prep starts ~0.3us in
            idx_all = cpool.tile([P, IDX_W], i16)
            w0 = TCH * P * DEG // 16
            nc.sync.dma_start(out=idx_all[:, 0:w0], in_=idxs[:, 0:w0])
            nc.sync.dma_start(out=idx_all[:, w0:], in_=idxs[:, w0:])
            msk_sb = cpool.tile([P, NT * DEG], u8)
            nc.sync.dma_start(out=msk_sb[:], in_=msk[:])
            xsT_sb = cpool.tile([C + 1, NSH_PAD], bf16)
            nc.sync.dma_start(out=xsT_sb[:], in_=xsT[:])
            v1_sb = cpool.tile([C + 1, C], bf16)
            nc.sync.dma_start(out=v1_sb[:], in_=v1[:])

            # A = x @ (W1-W2).T + b on the otherwise-idle PE (bias folded in
            # via the ones row of xsT / last row of v1).
            a_sb = cpool.tile([P, NT, C], bf16)
            for b in range(NT // MMB):
                ps = psum.tile([P, MMB, C], f32, tag="h")
                for t in range(MMB):
                    T = b * MMB + t
                    nc.tensor.matmul(
                        out=ps[:, t, :],
                        lhsT=xsT_sb[:, T * P : (T + 1) * P],
                        rhs=v1_sb[:],
                        start=True,
                        stop=True,
                    )
                nc.scalar.copy(out=a_sb[:, b * MMB : (b + 1) * MMB, :], in_=ps[:])

            NI = TCH * P * DEG
            tb = 0  # node-tile cursor
            for ch in range(NCHUNKS):
                s0, s1 = tb * DEG, (tb + TCH) * DEG  # slot range
                # position j lands at partition j%128, slot j//128; each slot
                # holds a 256B row-pair [even | odd]
                gath = gpool.tile([P, TCH * DEG, 2 * C], bf16, tag="gath")
                nc.gpsimd.dma_gather(
                    out_ap=gath[:],
                    in_ap=gpair[:],
                    idxs_ap=idx_all[:, tb * P * DEG // 16 : (tb + TCH) * P * DEG // 16],
                    num_idxs=NI,
                    num_idxs_reg=NI,
                    elem_size=2 * C,
                    transpose=False,
                    queue_num=0,
                    single_packet=False,
                )
                # parity half-select in place: the pair's even half becomes
                # the selected value (odd overwrites it where mask=1)
                nc.vector.copy_predicated(
                    out=gath[:, :, 0:C],
                    mask=msk_sb[:, s0:s1].to_broadcast([P, TCH * DEG, C]),
                    data=gath[:, :, C : 2 * C],
                )
                # contiguous-halves max tree over each dst's 16 slots (exact
                # in bf16: max just picks one value)
                s4 = gath[:].rearrange("p (t k) c -> p t k c", k=DEG)
                t8 = pool.tile([P, TCH, 8, C], bf16, tag="t8")
                nc.vector.tensor_tensor(
                    out=t8[:], in0=s4[:, :, 0:8, 0:C], in1=s4[:, :, 8:16, 0:C],
                    op=mybir.AluOpType.max,
                )
                t4 = pool.tile([P, TCH, 4, C], bf16, tag="t4")
                nc.vector.tensor_tensor(
                    out=t4[:], in0=t8[:, :, 0:4, :], in1=t8[:, :, 4:8, :],
                    op=mybir.AluOpType.max,
                )
                t2 = pool.tile([P, TCH, 2, C], bf16, tag="t2")
                nc.vector.tensor_tensor(
                    out=t2[:], in0=t4[:, :, 0:2, :], in1=t4[:, :, 2:4, :],
                    op=mybir.AluOpType.max,
                )
                m = pool.tile([P, TCH, C], bf16, tag="m")
                nc.vector.tensor_tensor(
                    out=m[:], in0=t2[:, :, 0, :], in1=t2[:, :, 1, :],
                    op=mybir.AluOpType.max,
                )
                s = pool.tile([P, TCH, C], bf16, tag="s")
                nc.vector.tensor_add(
                    out=s[:], in0=m[:], in1=a_sb[:, tb : tb + TCH, :]
                )
                o = pool.tile([P, TCH, C], bf16, tag="o")
                nc.vector.tensor_scalar_max(out=o[:], in0=s[:], scalar1=0.0)
                nc.sync.dma_start(
                    out=osh[:, :].rearrange("(p T) c -> p T c", p=P)[:, tb : tb + TCH, :],
                    in_=o[:],
                )
                tb += TCH
    nc.compile()
    return nc


def _glob_row(src):
    """Global permuted g-table row of source node id: the owning core's shard
    is written t-major (HBM row 49*(l%128) + l//128 for local id l)."""
    c = src // NSH
    l = src % NSH
    return c * NSH_PAD + (l % P) * NT + l // P


def _make_indices(src_rows):
    """src_rows: [NSH_PAD, DEG] int64 global table rows (pad rows = 0).
    Returns (idx, msk): pair-row ids in dma_gather's wrapped index layout
    [128, IDX_W] (chunk-major, position j of a chunk at [j%16, base+j//16],
    replicated 8x down partitions), and the odd-parity mask in dest layout
    [128, NT*DEG]. Gather position j of a chunk starting at node-tile tb
    covers node 128*(tb + j//128//DEG) + j%128, slot k = (j//128)%DEG."""
    idx = np.zeros((P, IDX_W), dtype=np.int16)
    msk = np.zeros((P, NT * DEG), dtype=np.uint8)
    tb = 0
    for TC in CHUNK_T:
        q = src_rows[tb * P : (tb + TC) * P].reshape(TC, P, DEG)
        posval = np.transpose(q, (0, 2, 1)).reshape(TC * P * DEG)  # j = (t k p)
        pair = (posval >> 1).astype(np.int16)
        par = (posval & 1).astype(np.uint8)
        ni = TC * P * DEG
        a = np.swapaxes(pair.reshape(ni // 16, 16), 0, 1)  # [16, ni/16]
        idx[:, tb * P * DEG // 16 : (tb + TC) * P * DEG // 16] = np.tile(a, (8, 1))
        m = np.swapaxes(par.reshape(TC * DEG, P), 0, 1)  # [p, slots]
        msk[:, tb * DEG : (tb + TC) * DEG] = m
        tb += TC
    return idx, msk


def _numpy_fallback(x, edge_index, W, b):
    src, dst = edge_index[0], edge_index[1]
    V1 = W[:, :C] - W[:, C:]
    V2 = W[:, C:]
    A = x @ V1.T + b
    g = x @ V2.T
    out = np.full((x.shape[0], C), -np.inf, dtype=np.float32)
    msg = np.maximum(A[dst] + g[src], 0.0)
    np.maximum.at(out, dst, msg)
    return np.where(np.isneginf(out), 0.0, out).astype(np.float32)


def _run_spmd(nc, in_maps):
    # the shared axon device occasionally reports a transient
    # NRT_EXEC_UNIT_UNRECOVERABLE on a cold first launch; retry with backoff
    import time
    from concourse.bass_utils import run_bass_kernel_spmd

    last = None
    for wait in (0.0, 10.0, 30.0, 60.0):
        if wait:
            time.sleep(wait)
        try:
            return run_bass_kernel_spmd(nc, in_maps, core_ids=list(range(N_CORES)))
        except Exception as e:
            last = e
    raise last


def kernel(x, edge_index, edge_attr, W, b):
    import ml_dtypes

    bf16 = ml_dtypes.bfloat16

    x = np.ascontiguousarray(x, dtype=np.float32)
    edge_index = np.ascontiguousarray(edge_index, dtype=np.int32)
    W = np.ascontiguousarray(W, dtype=np.float32)
    b = np.ascontiguousarray(b, dtype=np.float32)

    expected_dst = np.repeat(np.arange(N_NODES, dtype=np.int32), DEG)
    if (
        x.shape != (N_NODES, C)
        or edge_index.shape != (2, N_NODES * DEG)
        or not np.array_equal(edge_index[1], expected_dst)
    ):
        return _numpy_fallback(x, edge_index, W, b)

    if "dense" not in _cache:
        _cache["dense"] = _build_dense()
    if "gather" not in _cache:
        _cache["gather"] = _build_gather()

    v1 = np.concatenate([(W[:, :C] - W[:, C:]).T, b[None, :]], 0).astype(bf16)
    v2 = np.concatenate([W[:, C:].T, np.zeros((1, C), np.float32)], 0).astype(bf16)

    # ---- Launch 1: node-parallel dense phase (g table) ----
    in1 = []
    xsTs = []
    for c in range(N_CORES):
        xs = np.zeros((NSH_PAD, C), dtype=np.float32)
        xs[:NSH] = x[c * NSH : (c + 1) * NSH]
        xsT = np.concatenate([xs.T, np.ones((1, NSH_PAD), np.float32)], 0).astype(bf16)
        xsTs.append(xsT)
        in1.append({"xsT": xsT, "v2": v2})
    r1 = _run_spmd(_cache["dense"], in1)

    gpair = np.ascontiguousarray(
        np.concatenate([r1.results[c]["gout"] for c in range(N_CORES)], 0).reshape(
            NPAIR, 2 * C
        )
    )

    # ---- Launch 2: edge-parallel gather + segment max ----
    src = edge_index[0]
    in2 = []
    for c in range(N_CORES):
        rows = np.zeros((NSH_PAD, DEG), dtype=np.int64)
        rows[:NSH] = _glob_row(
            src[c * NSH * DEG : (c + 1) * NSH * DEG].reshape(NSH, DEG).astype(np.int64)
        )
        idx, msk = _make_indices(rows)
        in2.append({"gpair": gpair, "idxs": idx, "msk": msk, "xsT": xsTs[c], "v1": v1})
    r2 = _run_spmd(_cache["gather"], in2)

    outs = []
    for c in range(N_CORES):
        osh = np.asarray(r2.results[c]["osh"]).astype(np.float32)  # rows r = 49p + T
        nodes = osh.reshape(P, NT, C).transpose(1, 0, 2).reshape(NSH_PAD, C)
        outs.append(nodes[:NSH])
    out = np.ascontiguousarray(np.concatenate(outs, 0), dtype=np.float32)
    _cache["last_results"] = (r1, r2)
    return out
